# revision 18
# baseline (speedup 1.0000x reference)
"""Trainium2 Bass kernel for CombinedRankingLoss (BCE + pairwise margin ranking).

Full inputs: logits/labels/weights [64, 1024, 1] f32. Output: scalar f32.

Strategy (data-parallel over batch, 8 cores x 8 batches):
  For batch b with scores v, labels y (0/1):
    a_i = v_i if y_i==1 else +BIG        (pos set, masked)
    b_j = v_j + MARGIN if y_j==0 else -BIG  (neg set, masked)
    pair_sum_b = sum_ij relu(b_j - a_i)  -- masked pairs contribute 0
  Device computes pair_sum_b per (batch, 128-row chunk) plus the BCE partial
  sum; host combines: counts, per-batch means, valid masking, final scalar.

  Per batch on device:
    - PE matmul (K=2, ones lhsT) broadcasts the masked b-row [1,1024] to
      [128,1024] PSUM while adding the -BIG neg-mask offset row.
    - 8 chunk ops (one per 128 candidates i): fused compare-reduce in ONE
      instruction per engine:
        ScalarE: activation(Relu, bias=-a_col, accum_out)   [reads PSUM]
        VectorE: tensor_scalar(add -a, max 0, accum_out)    [reads SBUF copy]
        GPSIMD:  tensor_scalar(add -a, max 0, accum_out)    [reads SBUF copy]
    - accum columns land in per-engine [128,66] tiles; ones-matmul reduces
      partitions -> [1,66] -> DMA out. col 64 = BCE partial.
"""
import sys
import numpy as np

sys.path.insert(0, "/opt/trn_rl_repo")

B, N = 64, 1024
N_CORES = 8
BLOC = B // N_CORES          # batches per core
NCH = N // 128               # 128-row chunks per batch
MARGIN = 0.5
# mask sentinel: must dominate |v|+margin (|v| <~ 5.5 for N(0,1) data) while
# keeping sums of 1024*BIG well inside f32 integer-exact range
BIG = 16.0

_CACHE = {}


def _patch_bass(bass):
    """Split multi-wait instructions (old walrus TPB_CTRL takes 1 wait)."""
    import json as _json
    if getattr(bass.Bass, "_wait_split_patched", False):
        return
    _orig = bass.Bass.to_json_bytes

    def _split(bir, limit=1):
        m = _json.loads(bir)
        for fn in m["functions"]:
            for bb in fn["blocks"]:
                out = []
                for i in bb.get("instructions", []):
                    si = i.get("sync_info") or {}
                    ow = si.get("on_wait") or []
                    if len(ow) > limit:
                        extra, keep = ow[:-limit], ow[-limit:]
                        for k, w in enumerate(extra):
                            out.append({
                                "debug": i.get("debug"), "engine": i["engine"],
                                "ins": [], "outs": [],
                                "name": i["name"] + f"_ws{k}",
                                "opcode": "NoOp",
                                "sync_info": {"on_wait": [w]},
                            })
                        si = dict(si)
                        si["on_wait"] = keep
                        i = dict(i)
                        i["sync_info"] = si
                    out.append(i)
                bb["instructions"] = out
        return _json.dumps(m).encode()

    bass.Bass.to_json_bytes = lambda self: _split(_orig(self))
    bass.Bass._wait_split_patched = True


def _engine_for(k):
    # Pool/GPSIMD rejects TensorScalarPtr on this walrus; split the 64 chunk
    # ops DVE:ACT ~ 9:7 (DVE is faster per op but also owns copies + prep)
    return "dve" if (k * 9) % 16 < 9 else "act"


def _build(bass, tile, mybir):
    f32 = mybir.dt.float32
    Alu = mybir.AluOpType
    Act = mybir.ActivationFunctionType

    nc = bass.Bass()
    v_d = nc.declare_dram_parameter("v", [BLOC, N], f32, isOutput=False)
    y_d = nc.declare_dram_parameter("y", [BLOC, N], f32, isOutput=False)
    w_d = nc.declare_dram_parameter("w", [BLOC, N], f32, isOutput=False)
    out_d = nc.declare_dram_parameter("out", [1, 66], f32, isOutput=True)

    with tile.TileContext(nc) as tc:
        with (
            tc.tile_pool(name="const", bufs=1) as const,
            tc.tile_pool(name="work", bufs=2) as work,
            tc.tile_pool(name="psum", bufs=2, space="PSUM") as psum,
            tc.tile_pool(name="psum1", bufs=1, space="PSUM") as psum1,
        ):
            # ---------- loads ----------
            v_rows = const.tile([BLOC, N], f32)
            y_rows = const.tile([BLOC, N], f32)
            nc.sync.dma_start(out=v_rows[:], in_=v_d[:])
            nc.sync.dma_start(out=y_rows[:], in_=y_d[:])
            # flat [128, 64] views for BCE
            v_flat = const.tile([128, 64], f32)
            y_flat = const.tile([128, 64], f32)
            w_flat = const.tile([128, 64], f32)
            nc.sync.dma_start(out=v_flat[:], in_=v_d[:].rearrange("a (c b) -> (a c) b", b=64))
            nc.sync.dma_start(out=y_flat[:], in_=y_d[:].rearrange("a (c b) -> (a c) b", b=64))
            nc.sync.dma_start(out=w_flat[:], in_=w_d[:].rearrange("a (c b) -> (a c) b", b=64))

            # ---------- constants ----------
            ones2 = const.tile([2, 128], f32)
            nc.vector.memset(ones2[:], 1.0)
            ones128 = const.tile([128, 1], f32)
            nc.vector.memset(ones128[:], 1.0)
            ident8 = const.tile([8, 8], f32)
            ones8 = const.tile([8, 8], f32)
            nc.vector.memset(ones8[:], 1.0)
            nc.gpsimd.affine_select(
                out=ident8[:], in_=ones8[:], pattern=[[1, 8]],
                compare_op=Alu.is_equal, fill=0.0, base=0, channel_multiplier=-1)
            # selector for the broadcast matmul: sel[k, b*128+m] = 1 iff k==b;
            # lhsT = sel[:, b*128:(b+1)*128] broadcasts tmask row b to all 128
            # output partitions. A second matmul accumulates the -BIG row.
            ones8n = const.tile([BLOC, N], f32)
            nc.gpsimd.memset(ones8n[:], 1.0)
            sel = const.tile([BLOC, N], f32)
            nc.gpsimd.affine_select(
                out=sel[:], in_=ones8n[:], pattern=[[-1, 8], [0, 128]],
                compare_op=Alu.is_equal, fill=0.0, base=0, channel_multiplier=1)
            negbig = const.tile([1, N], f32)
            nc.vector.memset(negbig[:], -BIG)

            # per-engine accumulators [128, 66]
            dve_acc = const.tile([128, 66], f32)
            act_acc = const.tile([128, 66], f32)
            nc.vector.memset(dve_acc[:], 0.0)
            nc.gpsimd.memset(act_acc[:], 0.0)

            # ---------- mask prep (rows) ----------
            # yneg = 1 - y
            yneg = work.tile([BLOC, N], f32)
            nc.vector.tensor_scalar(out=yneg[:], in0=y_rows[:], scalar1=-1.0,
                                    scalar2=1.0, op0=Alu.mult, op1=Alu.add)
            # tmask rows 0..7 = (v + BIG + MARGIN) * (1-y); broadcast matmul
            # then accumulates -BIG so bc[:, j] = v_j + M (neg j) | -BIG (pos j)
            tmask = const.tile([BLOC, N], f32)
            nc.vector.scalar_tensor_tensor(
                out=tmask[:], in0=v_rows[:], scalar=BIG + MARGIN,
                op0=Alu.add, op1=Alu.mult, in1=yneg[:])
            # t2 = (v - BIG) * y   (transposes to columns; negated+offset below)
            t2 = work.tile([BLOC, N], f32)
            nc.vector.scalar_tensor_tensor(
                out=t2[:], in0=v_rows[:], scalar=-BIG,
                op0=Alu.add, op1=Alu.mult, in1=y_rows[:])

            # ---------- transpose t2 -> na_cols ----------
            tp = psum1.tile([128, 64], f32)
            for c in range(NCH):
                nc.tensor.transpose(tp[:, c * 8:(c + 1) * 8],
                                    t2[:, c * 128:(c + 1) * 128], ident8[:])
            # na = -(t2) - BIG  ->  -v for pos rows, -BIG for non-pos rows
            # (ACT relu bias).  a_cols = t2 + BIG -> v | +BIG (DVE max operand).
            na_cols = const.tile([128, 64], f32)
            nc.vector.tensor_scalar(out=na_cols[:], in0=tp[:], scalar1=-1.0,
                                    scalar2=-BIG, op0=Alu.mult, op1=Alu.add)
            a_cols = const.tile([128, 64], f32)
            nc.vector.tensor_scalar(out=a_cols[:], in0=tp[:], scalar1=BIG,
                                    scalar2=None, op0=Alu.add)

            # ---------- BCE (flat) ----------
            # softplus(v) = ln(1 + exp(v)); exp+ln+relu share one ACT table set
            # (v in [-5, 5] so exp(v) <= ~150, no overflow)
            sp = work.tile([128, 64], f32)
            nc.scalar.activation(out=sp[:], in_=v_flat[:], func=Act.Exp)
            nc.vector.tensor_scalar(out=sp[:], in0=sp[:], scalar1=1.0, scalar2=None,
                                    op0=Alu.add)
            nc.scalar.activation(out=sp[:], in_=sp[:], func=Act.Ln)
            xy = work.tile([128, 64], f32)
            nc.vector.tensor_tensor(out=xy[:], in0=v_flat[:], in1=y_flat[:], op=Alu.mult)
            nc.vector.tensor_tensor(out=xy[:], in0=sp[:], in1=xy[:], op=Alu.subtract)
            bce_scr = work.tile([128, 64], f32)
            nc.vector.scalar_tensor_tensor(
                out=bce_scr[:], in0=xy[:], scalar=1.0, op0=Alu.mult,
                op1=Alu.mult, in1=w_flat[:], accum_out=dve_acc[:, 64:65])

            # ---------- main pairwise loop ----------
            scr_dve = const.tile([128, N], f32)
            scr_act = const.tile([128, N], f32)
            k = 0
            for b in range(BLOC):
                bcp = psum.tile([128, N], f32, tag="bcp")
                for h in range(2):
                    hs = slice(h * 512, (h + 1) * 512)
                    nc.tensor.matmul(bcp[:, hs], sel[:, b * 128:(b + 1) * 128],
                                     tmask[:, hs], start=True, stop=False)
                    nc.tensor.matmul(bcp[:, hs], ones2[0:1, :], negbig[:, hs],
                                     start=False, stop=True)
                bcs = work.tile([128, N], f32, tag="bcs")
                nc.vector.tensor_copy(out=bcs[:], in_=bcp[:])
                for c in range(NCH):
                    col = b * 8 + c
                    cidx = c * 8 + b
                    eng = _engine_for(k)
                    k += 1
                    if eng == "act":
                        # relu(bc + (-a)) summed over j (accum_out)
                        nc.scalar.activation(
                            out=scr_act[:], in_=bcp[:], func=Act.Relu,
                            bias=na_cols[:, cidx:cidx + 1], scale=1.0,
                            accum_out=act_acc[:, col:col + 1])
                    else:
                        # DVE tensor_scalar+accum: op0 elementwise, op1 = REDUCER.
                        # acc = sum_j max(bc_j, a_p); host subtracts N*a_p.
                        nc.vector.tensor_scalar(
                            out=scr_dve[:], in0=bcs[:],
                            scalar1=a_cols[:, cidx:cidx + 1], scalar2=None,
                            op0=Alu.max, op1=Alu.add,
                            accum_out=dve_acc[:, col:col + 1])

            # ---------- final partition reduction ----------
            fin = psum1.tile([1, 66], f32)
            nc.tensor.matmul(fin[:], ones128[:], dve_acc[:], start=True, stop=False)
            nc.tensor.matmul(fin[:], ones128[:], act_acc[:], start=False, stop=True)
            outs = const.tile([1, 66], f32)
            nc.vector.tensor_copy(out=outs[:], in_=fin[:])
            nc.sync.dma_start(out=out_d[:], in_=outs[:])

    return nc


def _get_nc():
    if "nc" not in _CACHE:
        import concourse.bass as bass
        import concourse.tile as tile
        from concourse import mybir
        _patch_bass(bass)
        _CACHE["nc"] = _build(bass, tile, mybir)
    return _CACHE["nc"]


def kernel(logits, labels, weights):
    from concourse.bass_utils import run_bass_kernel_spmd

    nc = _get_nc()
    v = np.ascontiguousarray(logits.reshape(B, N), dtype=np.float32)
    y = np.ascontiguousarray(labels.reshape(B, N), dtype=np.float32)
    w = np.ascontiguousarray(weights.reshape(B, N), dtype=np.float32)

    in_maps = [
        {"v": v[c * BLOC:(c + 1) * BLOC],
         "y": y[c * BLOC:(c + 1) * BLOC],
         "w": w[c * BLOC:(c + 1) * BLOC]}
        for c in range(N_CORES)
    ]
    res = run_bass_kernel_spmd(nc, in_maps, list(range(N_CORES)))

    # ---------- host combine ----------
    # DVE-assigned (b, c) columns hold sum_p sum_j max(bc_j, a_p); subtract
    # the N * sum_p(a_masked) correction to recover sum relu(bc_j - a_p).
    dve_col = np.array([_engine_for(k) == "dve" for k in range(BLOC * NCH)])
    a_masked = np.where(y == 1.0, v, BIG).astype(np.float64)          # [B, N]
    chunk_a = a_masked.reshape(B, NCH, 128).sum(axis=2)               # [B, NCH]

    bce_sum = 0.0
    pair_sums = np.zeros(B, dtype=np.float64)
    for c in range(N_CORES):
        r = np.asarray(res.results[c]["out"]).reshape(66).astype(np.float64)
        bce_sum += float(r[64])
        cols = r[:64].reshape(BLOC, NCH)                              # [b, c]
        corr = np.where(dve_col.reshape(BLOC, NCH),
                        N * chunk_a[c * BLOC:(c + 1) * BLOC], 0.0)
        pair_sums[c * BLOC:(c + 1) * BLOC] = (cols - corr).sum(axis=1)

    n_pos = y.sum(axis=1).astype(np.float64)              # exact small ints
    n_neg = N - n_pos
    n_pairs = n_pos * n_neg
    valid = n_pairs > 0
    per_batch_mean = np.where(valid, pair_sums / np.maximum(n_pairs, 1.0), 0.0)
    valid_count = valid.sum()
    rank_loss = per_batch_mean.sum() / valid_count if valid_count > 0 else 0.0
    bce_loss = bce_sum / (B * N)
    return np.float32(bce_loss + rank_loss)


# revision 20
# speedup vs baseline: 1.5688x; 1.5688x over previous
"""Trainium2 Bass kernel for CombinedRankingLoss (BCE + pairwise margin ranking).

Full inputs: logits/labels/weights [64, 1024, 1] f32. Output: scalar f32.

Data-parallel over batch: 8 cores x 8 batches. The pairwise term
    T_b = sum_{i in pos} sum_{j in neg} relu((v_j + M) - v_i)
is computed per batch with the candidates PERMUTED on host (the loss is
invariant to per-batch candidate order):
  - a side (i): pos-compacted, padded to KA=640 with +BIG  (sent negated)
  - b side (j): neg-compacted (v+M), padded to KB=640 with -BIG, and sent
    as a bf16 hi/lo pair whose sum reconstructs f32 to ~1e-4
Masked pads contribute exactly 0 through the relu.

Per batch on device:
  - PE: one [16,128] bf16 selector matmul broadcasts (hi_b + lo_b) rows to a
    [128, KB] f32 PSUM tile.
  - 5 chunk ops (128 pos candidates each) fused compare+row-reduce, one
    instruction each, split across ScalarE and VectorE (both read PSUM):
      ScalarE: activation(Relu, bias=-a_col, accum_out)  -> sum_j relu(b_j-a_i)
      VectorE: tensor_scalar(op0=max a_col, op1=add-as-REDUCER, accum_out)
               -> sum_j max(b_j, a_i); host subtracts KB*a_i (exact identity
               sum_j max(b_j,a)-KB*a = sum_j relu(b_j-a))
  - BCE partials on flat [128,64] tiles (softplus = ln(1+exp), one ACT table
    set shared with Relu).
  - ones-matmul reduces accumulator partitions -> [1,48] -> DMA out.
Host: per-batch counts/means/valid handling + final scalar (f64).
"""
import sys
import numpy as np

sys.path.insert(0, "/opt/trn_rl_repo")

B, N = 64, 1024
N_CORES = 8
BLOC = B // N_CORES          # batches per core
KA = 640                     # padded pos-count (i side); 5 chunks of 128
KB = 640                     # padded neg-count (j side, free dim)
NCH = KA // 128
MARGIN = 0.5
BIG = 16.0                   # mask sentinel; dominates |v|+margin (|v|<~5.5)
NACC = NCH * BLOC            # 40 accumulator columns
NOUT = 48

_CACHE = {}


def _patch_bass(bass):
    """Split multi-wait instructions (old walrus TPB_CTRL takes 1 wait)."""
    import json as _json
    if getattr(bass.Bass, "_wait_split_patched", False):
        return
    _orig = bass.Bass.to_json_bytes

    def _split(bir, limit=1):
        m = _json.loads(bir)
        for fn in m["functions"]:
            for bb in fn["blocks"]:
                out = []
                for i in bb.get("instructions", []):
                    si = i.get("sync_info") or {}
                    ow = si.get("on_wait") or []
                    if len(ow) > limit:
                        extra, keep = ow[:-limit], ow[-limit:]
                        for k, w in enumerate(extra):
                            out.append({
                                "debug": i.get("debug"), "engine": i["engine"],
                                "ins": [], "outs": [],
                                "name": i["name"] + f"_ws{k}",
                                "opcode": "NoOp",
                                "sync_info": {"on_wait": [w]},
                            })
                        si = dict(si)
                        si["on_wait"] = keep
                        i = dict(i)
                        i["sync_info"] = si
                    out.append(i)
                bb["instructions"] = out
        return _json.dumps(m).encode()

    bass.Bass.to_json_bytes = lambda self: _split(_orig(self))
    bass.Bass._wait_split_patched = True


def _engine_for(k):
    # split the 40 chunk ops DVE:ACT ~ 22:18
    return "dve" if (k * 22) % 40 < 22 else "act"


def _build(bass, tile, mybir):
    f32 = mybir.dt.float32
    bf16 = mybir.dt.bfloat16
    Alu = mybir.AluOpType
    Act = mybir.ActivationFunctionType

    nc = bass.Bass()
    na_d = nc.declare_dram_parameter("na", [BLOC, KA], f32, isOutput=False)
    b2_d = nc.declare_dram_parameter("b2", [16, KB], bf16, isOutput=False)
    v_d = nc.declare_dram_parameter("v", [128, 64], f32, isOutput=False)
    y_d = nc.declare_dram_parameter("y", [128, 64], f32, isOutput=False)
    w_d = nc.declare_dram_parameter("w", [128, 64], f32, isOutput=False)
    out_d = nc.declare_dram_parameter("out", [1, NOUT], f32, isOutput=True)

    with tile.TileContext(nc) as tc:
        with (
            tc.tile_pool(name="const", bufs=1) as const,
            tc.tile_pool(name="work", bufs=2) as work,
            tc.tile_pool(name="psum", bufs=3, space="PSUM") as psum,
            tc.tile_pool(name="psum1", bufs=1, space="PSUM") as psum1,
        ):
            # ---------- loads ----------
            na_rows = const.tile([BLOC, KA], f32)
            nc.sync.dma_start(out=na_rows[:], in_=na_d[:])
            b2 = const.tile([16, KB], bf16)
            nc.sync.dma_start(out=b2[:], in_=b2_d[:])
            v_flat = const.tile([128, 64], f32)
            y_flat = const.tile([128, 64], f32)
            w_flat = const.tile([128, 64], f32)
            nc.sync.dma_start(out=v_flat[:], in_=v_d[:])
            nc.sync.dma_start(out=y_flat[:], in_=y_d[:])
            nc.sync.dma_start(out=w_flat[:], in_=w_d[:])

            # ---------- constants ----------
            ones128 = const.tile([128, 1], f32)
            nc.vector.memset(ones128[:], 1.0)
            ident8 = const.tile([8, 8], f32)
            ones8 = const.tile([8, 8], f32)
            nc.vector.memset(ones8[:], 1.0)
            nc.gpsimd.affine_select(
                out=ident8[:], in_=ones8[:], pattern=[[1, 8]],
                compare_op=Alu.is_equal, fill=0.0, base=0, channel_multiplier=-1)
            # selector weights: sel16[k, b*128+m] = 1 iff k==b or k==b+8
            # (built as two bands on gpsimd, then summed)
            ones16 = const.tile([16, N], bf16)
            nc.gpsimd.memset(ones16[:], 1.0)
            sel_hi = const.tile([16, N], bf16)
            nc.gpsimd.affine_select(
                out=sel_hi[:], in_=ones16[:], pattern=[[-1, 8], [0, 128]],
                compare_op=Alu.is_equal, fill=0.0, base=0, channel_multiplier=1)
            sel_lo = const.tile([16, N], bf16)
            nc.gpsimd.affine_select(
                out=sel_lo[:], in_=ones16[:], pattern=[[-1, 8], [0, 128]],
                compare_op=Alu.is_equal, fill=0.0, base=-8, channel_multiplier=1)
            sel16 = const.tile([16, N], bf16)
            nc.gpsimd.tensor_tensor(out=sel16[:], in0=sel_hi[:], in1=sel_lo[:],
                                    op=Alu.add)

            # accumulators
            dve_acc = const.tile([128, NOUT], f32)
            act_acc = const.tile([128, NOUT], f32)
            nc.vector.memset(dve_acc[:], 0.0)
            nc.gpsimd.memset(act_acc[:], 0.0)

            # ---------- a-columns via PE transpose ----------
            tp = psum1.tile([128, NCH * 8], f32)
            for c in range(NCH):
                nc.tensor.transpose(tp[:, c * 8:(c + 1) * 8],
                                    na_rows[:, c * 128:(c + 1) * 128], ident8[:])
            na_cols = const.tile([128, NCH * 8], f32)   # -a  (ACT bias)
            nc.vector.tensor_copy(out=na_cols[:], in_=tp[:])
            a_cols = const.tile([128, NCH * 8], f32)    # +a  (DVE max operand)
            nc.vector.tensor_scalar(out=a_cols[:], in0=tp[:], scalar1=-1.0,
                                    scalar2=None, op0=Alu.mult)

            # ---------- BCE (flat [128,64]) ----------
            sp = work.tile([128, 64], f32)
            nc.scalar.activation(out=sp[:], in_=v_flat[:], func=Act.Exp)
            nc.vector.tensor_scalar(out=sp[:], in0=sp[:], scalar1=1.0,
                                    scalar2=None, op0=Alu.add)
            nc.scalar.activation(out=sp[:], in_=sp[:], func=Act.Ln)
            xy = work.tile([128, 64], f32)
            nc.vector.tensor_tensor(out=xy[:], in0=v_flat[:], in1=y_flat[:], op=Alu.mult)
            nc.vector.tensor_tensor(out=xy[:], in0=sp[:], in1=xy[:], op=Alu.subtract)
            bce_scr = work.tile([128, 64], f32)
            nc.vector.scalar_tensor_tensor(
                out=bce_scr[:], in0=xy[:], scalar=1.0, op0=Alu.mult,
                op1=Alu.mult, in1=w_flat[:], accum_out=dve_acc[:, NACC:NACC + 1])

            # ---------- main pairwise loop ----------
            scr_dve = const.tile([128, KB], f32)
            scr_act = const.tile([128, KB], f32)
            k = 0
            for b in range(BLOC):
                bcp = psum.tile([128, KB], f32, tag="bcp")
                lhsT = sel16[:, b * 128:(b + 1) * 128]
                nc.tensor.matmul(bcp[:, 0:512], lhsT, b2[:, 0:512],
                                 start=True, stop=True)
                nc.tensor.matmul(bcp[:, 512:KB], lhsT, b2[:, 512:KB],
                                 start=True, stop=True)
                for c in range(NCH):
                    col = b * NCH + c
                    cidx = c * 8 + b
                    eng = _engine_for(k)
                    k += 1
                    if eng == "act":
                        nc.scalar.activation(
                            out=scr_act[:], in_=bcp[:], func=Act.Relu,
                            bias=na_cols[:, cidx:cidx + 1], scale=1.0,
                            accum_out=act_acc[:, col:col + 1])
                    else:
                        nc.vector.tensor_scalar(
                            out=scr_dve[:], in0=bcp[:],
                            scalar1=a_cols[:, cidx:cidx + 1], scalar2=None,
                            op0=Alu.max, op1=Alu.add,
                            accum_out=dve_acc[:, col:col + 1])

            # ---------- final partition reduction ----------
            fin = psum1.tile([1, NOUT], f32)
            nc.tensor.matmul(fin[:], ones128[:], dve_acc[:], start=True, stop=False)
            nc.tensor.matmul(fin[:], ones128[:], act_acc[:], start=False, stop=True)
            outs = const.tile([1, NOUT], f32)
            nc.vector.tensor_copy(out=outs[:], in_=fin[:])
            nc.sync.dma_start(out=out_d[:], in_=outs[:])

    return nc


def _get_nc():
    if "nc" not in _CACHE:
        import concourse.bass as bass
        import concourse.tile as tile
        from concourse import mybir
        _patch_bass(bass)
        _CACHE["nc"] = _build(bass, tile, mybir)
    return _CACHE["nc"]


def _prep_core(v, y):
    """Compact one core's batches: returns na [BLOC,KA] f32, b2 [16,KB] bf16,
    a_pad [BLOC,KA] f64 (for the DVE correction), overflow list."""
    import ml_dtypes
    na = np.full((BLOC, KA), -BIG, dtype=np.float32)
    b_pad = np.full((BLOC, KB), -BIG, dtype=np.float32)
    overflow = []
    for r in range(BLOC):
        pos = v[r][y[r] == 1.0]
        neg = v[r][y[r] == 0.0] + np.float32(MARGIN)
        if len(pos) > KA or len(neg) > KB:
            overflow.append(r)   # leave na row at -BIG => device contributes 0
            continue
        na[r, :len(pos)] = -pos
        b_pad[r, :len(neg)] = neg
    hi = b_pad.astype(ml_dtypes.bfloat16)
    lo = (b_pad - hi.astype(np.float32)).astype(ml_dtypes.bfloat16)
    b2 = np.concatenate([hi, lo], axis=0)           # [16, KB]
    b_eff = hi.astype(np.float64) + lo.astype(np.float64)
    return na, b2, -na.astype(np.float64), b_eff, overflow


def make_in_maps(v, y, w):
    in_maps, a_pads, overflows = [], [], []
    for c in range(N_CORES):
        sl = slice(c * BLOC, (c + 1) * BLOC)
        na, b2, a_pad, b_eff, ovf = _prep_core(v[sl], y[sl])
        a_pads.append(a_pad)
        overflows.append(ovf)
        in_maps.append({
            "na": na, "b2": b2,
            "v": v[sl].reshape(128, 64),
            "y": y[sl].reshape(128, 64),
            "w": w[sl].reshape(128, 64),
        })
    return in_maps, a_pads, overflows


def kernel(logits, labels, weights):
    from concourse.bass_utils import run_bass_kernel_spmd

    nc = _get_nc()
    v = np.ascontiguousarray(logits.reshape(B, N), dtype=np.float32)
    y = np.ascontiguousarray(labels.reshape(B, N), dtype=np.float32)
    w = np.ascontiguousarray(weights.reshape(B, N), dtype=np.float32)

    in_maps, a_pads, overflows = make_in_maps(v, y, w)
    res = run_bass_kernel_spmd(nc, in_maps, list(range(N_CORES)))

    # ---------- host combine ----------
    dve_col = np.array([_engine_for(k) == "dve" for k in range(NACC)])
    bce_sum = 0.0
    pair_sums = np.zeros(B, dtype=np.float64)
    for c in range(N_CORES):
        r = np.asarray(res.results[c]["out"]).reshape(NOUT).astype(np.float64)
        bce_sum += float(r[NACC])
        cols = r[:NACC].reshape(BLOC, NCH)
        chunk_a = a_pads[c].reshape(BLOC, NCH, 128).sum(axis=2)
        corr = np.where(dve_col.reshape(BLOC, NCH), KB * chunk_a, 0.0)
        pair_sums[c * BLOC:(c + 1) * BLOC] = (cols - corr).sum(axis=1)
        for rloc in overflows[c]:
            b = c * BLOC + rloc
            pos = v[b][y[b] == 1.0].astype(np.float64)
            neg = v[b][y[b] == 0.0].astype(np.float64) + MARGIN
            d = neg[None, :] - pos[:, None]
            pair_sums[b] = np.maximum(d, 0.0).sum()

    n_pos = y.sum(axis=1).astype(np.float64)
    n_neg = N - n_pos
    n_pairs = n_pos * n_neg
    valid = n_pairs > 0
    per_batch_mean = np.where(valid, pair_sums / np.maximum(n_pairs, 1.0), 0.0)
    valid_count = valid.sum()
    rank_loss = per_batch_mean.sum() / valid_count if valid_count > 0 else 0.0
    bce_loss = bce_sum / (B * N)
    return np.float32(bce_loss + rank_loss)


# revision 24
# speedup vs baseline: 1.5778x; 1.0057x over previous
"""Trainium2 Bass kernel for CombinedRankingLoss (BCE + pairwise margin ranking).

Full inputs: logits/labels/weights [64, 1024, 1] f32. Output: scalar f32.

Data-parallel over batch: 8 cores x 8 batches. The pairwise term
    T_b = sum_{i in pos} sum_{j in neg} relu((v_j + M) - v_i)
is computed per batch with the candidates PERMUTED on host (the loss is
invariant to per-batch candidate order):
  - a side (i): pos-compacted, padded to KA=640 with +BIG  (sent negated)
  - b side (j): neg-compacted (v+M), padded to KB=640 with -BIG, and sent
    as a bf16 hi/lo pair whose sum reconstructs f32 to ~1e-4
Masked pads contribute exactly 0 through the relu.

Per batch on device:
  - PE: one [16,128] bf16 selector matmul broadcasts (hi_b + lo_b) rows to a
    [128, KB] f32 PSUM tile.
  - 5 chunk ops (128 pos candidates each) fused compare+row-reduce, one
    instruction each, split across ScalarE and VectorE (both read PSUM):
      ScalarE: activation(Relu, bias=-a_col, accum_out)  -> sum_j relu(b_j-a_i)
      VectorE: tensor_scalar(op0=max a_col, op1=add-as-REDUCER, accum_out)
               -> sum_j max(b_j, a_i); host subtracts KB*a_i (exact identity
               sum_j max(b_j,a)-KB*a = sum_j relu(b_j-a))
  - BCE partials on flat [128,64] tiles (softplus = ln(1+exp), one ACT table
    set shared with Relu).
  - ones-matmul reduces accumulator partitions -> [1,48] -> DMA out.
Host: per-batch counts/means/valid handling + final scalar (f64).
"""
import sys
import numpy as np

sys.path.insert(0, "/opt/trn_rl_repo")

B, N = 64, 1024
N_CORES = 8
BLOC = B // N_CORES          # batches per core
KA = 640                     # padded pos-count (i side); 5 chunks of 128
KB = 640                     # padded neg-count (j side, free dim)
NCH = KA // 128
MARGIN = 0.5
BIG = 16.0                   # mask sentinel; dominates |v|+margin (|v|<~5.5)
NACC = NCH * BLOC            # 40 accumulator columns
NOUT = 48

_CACHE = {}


def _patch_bass(bass):
    """Split multi-wait instructions (old walrus TPB_CTRL takes 1 wait)."""
    import json as _json
    if getattr(bass.Bass, "_wait_split_patched", False):
        return
    _orig = bass.Bass.to_json_bytes

    def _split(bir, limit=1):
        m = _json.loads(bir)
        for fn in m["functions"]:
            for bb in fn["blocks"]:
                out = []
                for i in bb.get("instructions", []):
                    si = i.get("sync_info") or {}
                    ow = si.get("on_wait") or []
                    if len(ow) > limit:
                        extra, keep = ow[:-limit], ow[-limit:]
                        for k, w in enumerate(extra):
                            out.append({
                                "debug": i.get("debug"), "engine": i["engine"],
                                "ins": [], "outs": [],
                                "name": i["name"] + f"_ws{k}",
                                "opcode": "NoOp",
                                "sync_info": {"on_wait": [w]},
                            })
                        si = dict(si)
                        si["on_wait"] = keep
                        i = dict(i)
                        i["sync_info"] = si
                    out.append(i)
                bb["instructions"] = out
        return _json.dumps(m).encode()

    bass.Bass.to_json_bytes = lambda self: _split(_orig(self))
    bass.Bass._wait_split_patched = True


def _engine_for(k):
    # split the 40 chunk ops DVE:ACT ~ 16:24 (DVE pays a pipe-DRAIN per op)
    return "dve" if (k * 16) % 40 < 16 else "act"


def _build(bass, tile, mybir):
    f32 = mybir.dt.float32
    bf16 = mybir.dt.bfloat16
    Alu = mybir.AluOpType
    Act = mybir.ActivationFunctionType

    nc = bass.Bass()
    na_d = nc.declare_dram_parameter("na", [BLOC, KA], f32, isOutput=False)
    b2_d = nc.declare_dram_parameter("b2", [16, KB], bf16, isOutput=False)
    v_d = nc.declare_dram_parameter("v", [128, 64], f32, isOutput=False)
    y_d = nc.declare_dram_parameter("y", [128, 64], f32, isOutput=False)
    w_d = nc.declare_dram_parameter("w", [128, 64], f32, isOutput=False)
    sel_d = nc.declare_dram_parameter("sel", [16, N], bf16, isOutput=False)
    id8_d = nc.declare_dram_parameter("id8", [8, 8], f32, isOutput=False)
    out_d = nc.declare_dram_parameter("out", [1, NOUT], f32, isOutput=True)

    with tile.TileContext(nc) as tc:
        with (
            tc.tile_pool(name="const", bufs=1) as const,
            tc.tile_pool(name="work", bufs=2) as work,
            tc.tile_pool(name="psum", bufs=3, space="PSUM") as psum,
            tc.tile_pool(name="psum1", bufs=1, space="PSUM") as psum1,
        ):
            # ---------- loads ----------
            na_rows = const.tile([BLOC, KA], f32)
            nc.sync.dma_start(out=na_rows[:], in_=na_d[:])
            b2 = const.tile([16, KB], bf16)
            nc.sync.dma_start(out=b2[:], in_=b2_d[:])
            v_flat = const.tile([128, 64], f32)
            y_flat = const.tile([128, 64], f32)
            w_flat = const.tile([128, 64], f32)
            nc.sync.dma_start(out=v_flat[:], in_=v_d[:])
            nc.sync.dma_start(out=y_flat[:], in_=y_d[:])
            nc.sync.dma_start(out=w_flat[:], in_=w_d[:])

            # ---------- constants (selector/identity DMA'd from host) ----------
            ones128 = const.tile([128, 1], f32)
            nc.vector.memset(ones128[:], 1.0)
            ident8 = const.tile([8, 8], f32)
            nc.sync.dma_start(out=ident8[:], in_=id8_d[:])
            # sel16[k, b*128+m] = 1 iff k==b or k==b+8
            sel16 = const.tile([16, N], bf16)
            nc.sync.dma_start(out=sel16[:], in_=sel_d[:])

            # accumulators
            dve_acc = const.tile([128, NOUT], f32)
            act_acc = const.tile([128, NOUT], f32)
            nc.vector.memset(dve_acc[:], 0.0)
            nc.gpsimd.memset(act_acc[:], 0.0)

            # ---------- a-columns via PE transpose ----------
            tp = psum1.tile([128, NCH * 8], f32)
            for c in range(NCH):
                nc.tensor.transpose(tp[:, c * 8:(c + 1) * 8],
                                    na_rows[:, c * 128:(c + 1) * 128], ident8[:])
            na_cols = const.tile([128, NCH * 8], f32)   # -a  (ACT bias)
            nc.vector.tensor_copy(out=na_cols[:], in_=tp[:])
            a_cols = const.tile([128, NCH * 8], f32)    # +a  (DVE max operand)
            nc.vector.tensor_scalar(out=a_cols[:], in0=tp[:], scalar1=-1.0,
                                    scalar2=None, op0=Alu.mult)

            # ---------- BCE (flat [128,64]) ----------
            sp = work.tile([128, 64], f32)
            nc.scalar.activation(out=sp[:], in_=v_flat[:], func=Act.Exp)
            nc.vector.tensor_scalar(out=sp[:], in0=sp[:], scalar1=1.0,
                                    scalar2=None, op0=Alu.add)
            nc.scalar.activation(out=sp[:], in_=sp[:], func=Act.Ln)
            xy = work.tile([128, 64], f32)
            nc.vector.tensor_tensor(out=xy[:], in0=v_flat[:], in1=y_flat[:], op=Alu.mult)
            nc.vector.tensor_tensor(out=xy[:], in0=sp[:], in1=xy[:], op=Alu.subtract)
            bce_scr = work.tile([128, 64], f32)
            nc.vector.scalar_tensor_tensor(
                out=bce_scr[:], in0=xy[:], scalar=1.0, op0=Alu.mult,
                op1=Alu.mult, in1=w_flat[:], accum_out=dve_acc[:, NACC:NACC + 1])

            # ---------- main pairwise loop ----------
            scr_dve = const.tile([128, KB], f32)
            scr_act = const.tile([128, KB], f32)
            k = 0
            for b in range(BLOC):
                bcp = psum.tile([128, KB], f32, tag="bcp")
                lhsT = sel16[:, b * 128:(b + 1) * 128]
                nc.tensor.matmul(bcp[:, 0:512], lhsT, b2[:, 0:512],
                                 start=True, stop=True)
                nc.tensor.matmul(bcp[:, 512:KB], lhsT, b2[:, 512:KB],
                                 start=True, stop=True)
                for c in range(NCH):
                    col = b * NCH + c
                    cidx = c * 8 + b
                    eng = _engine_for(k)
                    k += 1
                    if eng == "act":
                        nc.scalar.activation(
                            out=scr_act[:], in_=bcp[:], func=Act.Relu,
                            bias=na_cols[:, cidx:cidx + 1], scale=1.0,
                            accum_out=act_acc[:, col:col + 1])
                    else:
                        nc.vector.tensor_scalar(
                            out=scr_dve[:], in0=bcp[:],
                            scalar1=a_cols[:, cidx:cidx + 1], scalar2=None,
                            op0=Alu.max, op1=Alu.add,
                            accum_out=dve_acc[:, col:col + 1])

            # ---------- final partition reduction ----------
            fin = psum1.tile([1, NOUT], f32)
            nc.tensor.matmul(fin[:], ones128[:], dve_acc[:], start=True, stop=False)
            nc.tensor.matmul(fin[:], ones128[:], act_acc[:], start=False, stop=True)
            outs = const.tile([1, NOUT], f32)
            nc.vector.tensor_copy(out=outs[:], in_=fin[:])
            nc.sync.dma_start(out=out_d[:], in_=outs[:])

    return nc


def _get_nc():
    if "nc" not in _CACHE:
        import concourse.bass as bass
        import concourse.tile as tile
        from concourse import mybir
        _patch_bass(bass)
        _CACHE["nc"] = _build(bass, tile, mybir)
    return _CACHE["nc"]


def _prep_core(v, y):
    """Compact one core's batches: returns na [BLOC,KA] f32, b2 [16,KB] bf16,
    a_pad [BLOC,KA] f64 (for the DVE correction), overflow list."""
    import ml_dtypes
    na = np.full((BLOC, KA), -BIG, dtype=np.float32)
    b_pad = np.full((BLOC, KB), -BIG, dtype=np.float32)
    overflow = []
    for r in range(BLOC):
        pos = v[r][y[r] == 1.0]
        neg = v[r][y[r] == 0.0] + np.float32(MARGIN)
        if len(pos) > KA or len(neg) > KB:
            overflow.append(r)   # leave na row at -BIG => device contributes 0
            continue
        na[r, :len(pos)] = -pos
        b_pad[r, :len(neg)] = neg
    hi = b_pad.astype(ml_dtypes.bfloat16)
    lo = (b_pad - hi.astype(np.float32)).astype(ml_dtypes.bfloat16)
    b2 = np.concatenate([hi, lo], axis=0)           # [16, KB]
    b_eff = hi.astype(np.float64) + lo.astype(np.float64)
    return na, b2, -na.astype(np.float64), b_eff, overflow


def _host_consts():
    import ml_dtypes
    sel = np.zeros((16, N), dtype=np.float32)
    for b in range(BLOC):
        sel[b, b * 128:(b + 1) * 128] = 1.0
        sel[b + 8, b * 128:(b + 1) * 128] = 1.0
    id8 = np.eye(8, dtype=np.float32)
    return sel.astype(ml_dtypes.bfloat16), id8


def make_in_maps(v, y, w):
    sel, id8 = _host_consts()
    in_maps, a_pads, overflows = [], [], []
    for c in range(N_CORES):
        sl = slice(c * BLOC, (c + 1) * BLOC)
        na, b2, a_pad, b_eff, ovf = _prep_core(v[sl], y[sl])
        a_pads.append(a_pad)
        overflows.append(ovf)
        in_maps.append({
            "na": na, "b2": b2,
            "v": v[sl].reshape(128, 64),
            "y": y[sl].reshape(128, 64),
            "w": w[sl].reshape(128, 64),
            "sel": sel, "id8": id8,
        })
    return in_maps, a_pads, overflows


def kernel(logits, labels, weights):
    from concourse.bass_utils import run_bass_kernel_spmd

    nc = _get_nc()
    v = np.ascontiguousarray(logits.reshape(B, N), dtype=np.float32)
    y = np.ascontiguousarray(labels.reshape(B, N), dtype=np.float32)
    w = np.ascontiguousarray(weights.reshape(B, N), dtype=np.float32)

    in_maps, a_pads, overflows = make_in_maps(v, y, w)
    res = run_bass_kernel_spmd(nc, in_maps, list(range(N_CORES)))

    # ---------- host combine ----------
    dve_col = np.array([_engine_for(k) == "dve" for k in range(NACC)])
    bce_sum = 0.0
    pair_sums = np.zeros(B, dtype=np.float64)
    for c in range(N_CORES):
        r = np.asarray(res.results[c]["out"]).reshape(NOUT).astype(np.float64)
        bce_sum += float(r[NACC])
        cols = r[:NACC].reshape(BLOC, NCH)
        chunk_a = a_pads[c].reshape(BLOC, NCH, 128).sum(axis=2)
        corr = np.where(dve_col.reshape(BLOC, NCH), KB * chunk_a, 0.0)
        pair_sums[c * BLOC:(c + 1) * BLOC] = (cols - corr).sum(axis=1)
        for rloc in overflows[c]:
            b = c * BLOC + rloc
            pos = v[b][y[b] == 1.0].astype(np.float64)
            neg = v[b][y[b] == 0.0].astype(np.float64) + MARGIN
            d = neg[None, :] - pos[:, None]
            pair_sums[b] = np.maximum(d, 0.0).sum()

    n_pos = y.sum(axis=1).astype(np.float64)
    n_neg = N - n_pos
    n_pairs = n_pos * n_neg
    valid = n_pairs > 0
    per_batch_mean = np.where(valid, pair_sums / np.maximum(n_pairs, 1.0), 0.0)
    valid_count = valid.sum()
    rank_loss = per_batch_mean.sum() / valid_count if valid_count > 0 else 0.0
    bce_loss = bce_sum / (B * N)
    return np.float32(bce_loss + rank_loss)
